# revision 26
# baseline (speedup 1.0000x reference)
"""Per-edge dot product kernel for Trainium2 (8 NeuronCores).

Computes out[e] = sum(h[src[e]] * h[dst[e]], axis=-1) for
h: [100000, 64] f32, src/dst: [1000000] int indices.

Bottleneck analysis (HW, bedrock image -- no extended GPSIMD ucode, so
dma_gather/ap_gather are unavailable and the only data-dependent gather
is core SWDGE indirect DMA):
  - indirect_dma_start supports ONE offset per partition -> max 128
    random rows per instruction, at a measured ~1.43us/instruction
    cadence on the GPSIMD engine (1.12us SWDGE exec + 0.31us fixed
    sequencer gap; microbenched as architectural, independent of sync
    structure).  Runtime ~= (#indirect DMAs) * 1.43us.
  - The original kernel needed 2*125000/128 = 1968 per core (2.82ms).

Design (src side gathered by PE, dst side dense-packed on GPSIMD):
  - Host sorts edges by src; core c takes the c-th contiguous 125k
    slice, so its src values span <=100 contiguous 128-row blocks of h.
  - Edges stay DENSELY packed in sorted order (no padding): chunk k =
    edges [128k, 128k+128).  Sorted + interior block runs >> 128 means
    each chunk's src rows lie in a 2-block window [b_k, b_k+1].
  - Src rows are gathered ON-CHIP by two PSUM-accumulated one-hot
    matmuls per chunk: onehot_w[128,128]^T @ window_w[128,64] (w=0,1),
    with onehot_w = is_equal(srclocal - 128w, iota) built on DVE.  The
    per-chunk 2-block windows are host-staged (block-granular slices of
    h, no per-edge host work) and streamed sequentially.
  - Dst rows use indirect DMA from a bf16 copy of h, round-robined
    over 4 SWDGE queues: 984 instructions/core (the dense minimum
    ceil(125000/128) rounded to supertiles).
  - dots = reduce_d(psum * dst_rows) on DVE; host inverse-permutes.

Measured: 2815us (baseline, 1968 indirect) -> 1527us (block-padded
schedule, 1032 indirect) -> 1421us (this version: 984 indirect at the
1.417us/instr cadence floor, ~2% non-gather overhead).
"""

import sys

import numpy as np

_TRN_REPO = "/opt/trn_rl_repo"
if _TRN_REPO not in sys.path:
    sys.path.insert(0, _TRN_REPO)

import ml_dtypes

N_NODES = 100000
N_EDGES = 1000000
D = 64
N_CORES = 8
E_CORE = N_EDGES // N_CORES   # 125000

SUPER = 8                     # chunks per supertile
ST = 123                      # supertiles per core
CHUNKS = ST * SUPER           # 984
E_PAD = CHUNKS * 128          # 125952 edge slots per core
B_MAX = 100                   # table blocks a core may span (seed-0 max 99)
N_SWDGE_Q = 4                 # parallel SWDGE dynamic queues

_PROGRAM_CACHE = {}


def _build_program():
    import concourse.bass as bass
    import concourse.tile as tile
    from concourse import bacc, mybir

    nc = bacc.Bacc(
        "TRN2",
        target_bir_lowering=False,
        debug=False,
        num_swdge_queues=N_SWDGE_Q,
        dynamic_dma_scratch_size=65536,
    )

    h_t = nc.dram_tensor("h_nodes", [N_NODES, D], mybir.dt.bfloat16, kind="ExternalInput")
    twin_t = nc.dram_tensor("tabwin", [ST, 128, SUPER * 2 * D], mybir.dt.bfloat16, kind="ExternalInput")
    srcl_t = nc.dram_tensor("srcl", [ST, SUPER * 128], mybir.dt.bfloat16, kind="ExternalInput")
    iota_lo_t = nc.dram_tensor("iota_lo", [128, SUPER * 128], mybir.dt.bfloat16, kind="ExternalInput")
    iota_hi_t = nc.dram_tensor("iota_hi", [128, SUPER * 128], mybir.dt.bfloat16, kind="ExternalInput")
    di_t = nc.dram_tensor("dst_idx", [128, CHUNKS], mybir.dt.int32, kind="ExternalInput")
    out_t = nc.dram_tensor("edot", [ST, 128, SUPER], mybir.dt.float32, kind="ExternalOutput")

    h_ap = h_t.ap()
    EW = SUPER * 128   # 1024 edges per supertile

    with tile.TileContext(nc) as tc:
        with (
            tc.tile_pool(name="const", bufs=1) as const_pool,
        ):
            iota_lo = const_pool.tile([128, EW], mybir.dt.bfloat16, tag="ilo")
            nc.sync.dma_start(out=iota_lo[:], in_=iota_lo_t.ap())
            iota_hi = const_pool.tile([128, EW], mybir.dt.bfloat16, tag="ihi")
            nc.sync.dma_start(out=iota_hi[:], in_=iota_hi_t.ap())

            # all dst indices up-front: one DMA, no per-supertile stalls
            di_all = const_pool.tile([128, CHUNKS], mybir.dt.int32, tag="diall")
            nc.sync.dma_start(out=di_all[:], in_=di_t.ap())

            with (
                tc.tile_pool(name="oh", bufs=4) as oh_pool,
                tc.tile_pool(name="tw", bufs=4) as tw_pool,
                tc.tile_pool(name="gat", bufs=6) as gat_pool,
                tc.tile_pool(name="res", bufs=3) as res_pool,
                tc.tile_pool(name="ps", bufs=5, space="PSUM") as ps_pool,
            ):
                for st in range(ST):
                    twin = tw_pool.tile([128, SUPER * 2 * D], mybir.dt.bfloat16, tag="tw")
                    nc.sync.dma_start(out=twin[:], in_=twin_t.ap()[st])

                    bc = oh_pool.tile([128, EW], mybir.dt.bfloat16, tag="bc")
                    nc.sync.dma_start(
                        out=bc[:],
                        in_=srcl_t.ap()[st][None, :].broadcast_to([128, EW]),
                    )
                    oh_lo = oh_pool.tile([128, EW], mybir.dt.bfloat16, tag="ohlo")
                    nc.vector.tensor_tensor(
                        out=oh_lo[:], in0=bc[:], in1=iota_lo[:],
                        op=mybir.AluOpType.is_equal,
                    )
                    oh_hi = oh_pool.tile([128, EW], mybir.dt.bfloat16, tag="ohhi")
                    nc.vector.tensor_tensor(
                        out=oh_hi[:], in0=bc[:], in1=iota_hi[:],
                        op=mybir.AluOpType.is_equal,
                    )

                    hd = gat_pool.tile([128, SUPER * D], mybir.dt.bfloat16, tag="hd")
                    for g in range(SUPER):
                        gi = nc.gpsimd.indirect_dma_start(
                            out=hd[:, g * D:(g + 1) * D],
                            out_offset=None,
                            in_=h_ap,
                            in_offset=bass.IndirectOffsetOnAxis(
                                ap=di_all[:, st * SUPER + g:st * SUPER + g + 1],
                                axis=0,
                            ),
                        )
                        gi.ins.queue = f"qPoolDynamic{(g % N_SWDGE_Q) or ''}"

                    psum = ps_pool.tile([128, SUPER * D], mybir.dt.float32, tag="ps")
                    for g in range(SUPER):
                        nc.tensor.matmul(
                            psum[:, g * D:(g + 1) * D],
                            oh_lo[:, g * 128:(g + 1) * 128],
                            twin[:, (g * 2) * D:(g * 2 + 1) * D],
                            start=True, stop=False,
                        )
                        nc.tensor.matmul(
                            psum[:, g * D:(g + 1) * D],
                            oh_hi[:, g * 128:(g + 1) * 128],
                            twin[:, (g * 2 + 1) * D:(g * 2 + 2) * D],
                            start=False, stop=True,
                        )

                    prod = gat_pool.tile([128, SUPER * D], mybir.dt.float32, tag="prod")
                    nc.vector.tensor_mul(out=prod[:], in0=psum[:], in1=hd[:])

                    dots = res_pool.tile([128, SUPER], mybir.dt.float32, tag="dots")
                    nc.vector.tensor_reduce(
                        out=dots[:],
                        in_=prod[:].rearrange("p (g d) -> p g d", d=D),
                        axis=mybir.AxisListType.X,
                        op=mybir.AluOpType.add,
                    )
                    nc.sync.dma_start(out=out_t.ap()[st], in_=dots[:])

    nc.compile()
    return nc


def _get_program():
    if "p" not in _PROGRAM_CACHE:
        _PROGRAM_CACHE["p"] = _build_program()
    return _PROGRAM_CACHE["p"]


def _prep_core(h_bf, src_s, dst_s, eid_s):
    """Dense slotting of one core's src-sorted edge slice.

    Edge j (sorted order) -> slot j; chunk k = slot//128.  Each chunk's
    src rows must lie in blocks [b_k, b_k+1] (asserted; holds because
    interior block runs >> 128).
    """
    n = len(src_s)
    base = int(src_s[0]) // 128 * 128
    local = src_s.astype(np.int64) - base
    if int(local[-1]) >= B_MAX * 128:
        raise RuntimeError(f"core spans > {B_MAX} blocks")

    # per-chunk window base block b_k (tail chunks repeat the last block)
    nchunk_real = -(-n // 128)
    b_arr = np.zeros(CHUNKS, dtype=np.int64)
    b_arr[:nchunk_real] = local[:: 128][:nchunk_real] >> 7
    b_arr[nchunk_real:] = b_arr[nchunk_real - 1]

    # srclocal relative to the chunk's window: in [0, 256)
    local_pad = np.zeros(E_PAD, dtype=np.int64)
    local_pad[:n] = local
    local_pad[n:] = b_arr[nchunk_real - 1] * 128  # padding -> window row 0
    rel = local_pad - np.repeat(b_arr, 128) * 128
    if rel.min() < 0 or rel.max() > 255:
        raise RuntimeError(f"chunk spans >2 blocks (rel range {rel.min()}..{rel.max()})")

    dst_slot = np.zeros(E_PAD, dtype=np.int32)
    dst_slot[:n] = dst_s.astype(np.int32)
    eid_slot = np.full(E_PAD, -1, dtype=np.int64)
    eid_slot[:n] = eid_s

    # host-staged per-chunk 2-block windows of h (block-granular slices)
    hloc = np.zeros((B_MAX * 128 + 256, D), dtype=ml_dtypes.bfloat16)
    hi = min(base + B_MAX * 128, N_NODES)
    hloc[: hi - base] = h_bf[base:hi]
    row_idx = b_arr[:, None] * 128 + np.arange(256)[None, :]   # [CHUNKS, 256]
    win = hloc[row_idx]                                        # [CHUNKS, 256, D]
    # device layout [ST, 128 p, SUPER*2*D]: partition p holds, for each
    # chunk c and window w, row (b+w*128+p) -> win[c, w*128+p, :]
    tabwin = np.ascontiguousarray(
        win.reshape(ST, SUPER, 2, 128, D)        # [st, c, w, p, d]
        .transpose(0, 3, 1, 2, 4)                # [st, p, c, w, d]
        .reshape(ST, 128, SUPER * 2 * D)
    )

    srcl = np.ascontiguousarray(
        rel.astype(np.float32).reshape(ST, SUPER * 128).astype(ml_dtypes.bfloat16)
    )
    dst_in = np.ascontiguousarray(dst_slot.reshape(CHUNKS, 128).T)
    return (
        {"tabwin": tabwin, "srcl": srcl, "dst_idx": dst_in},
        eid_slot,
    )


def _run(h, src, dst, trace=False):
    from concourse.bass_utils import run_bass_kernel_spmd

    h = np.ascontiguousarray(np.asarray(h, dtype=np.float32))
    src = np.asarray(src).astype(np.int64)
    dst = np.asarray(dst).astype(np.int64)

    h_bf = h.astype(ml_dtypes.bfloat16)
    perm = np.argsort(src, kind="stable")
    src_s = src[perm]
    dst_s = dst[perm]

    iota_base = np.tile(np.arange(128, dtype=np.float32)[:, None], (1, SUPER * 128))
    iota_lo = np.ascontiguousarray(iota_base.astype(ml_dtypes.bfloat16))
    iota_hi = np.ascontiguousarray((iota_base + 128).astype(ml_dtypes.bfloat16))

    in_maps = []
    eid_slots = []
    for c in range(N_CORES):
        sl = slice(c * E_CORE, (c + 1) * E_CORE)
        m, eid_slot = _prep_core(h_bf, src_s[sl], dst_s[sl], perm[sl])
        m["h_nodes"] = h_bf
        m["iota_lo"] = iota_lo
        m["iota_hi"] = iota_hi
        in_maps.append(m)
        eid_slots.append(eid_slot)

    nc = _get_program()
    res = run_bass_kernel_spmd(nc, in_maps, list(range(N_CORES)), trace=trace)

    out = np.empty(N_EDGES, dtype=np.float32)
    for c in range(N_CORES):
        dots = np.asarray(res.results[c]["edot"])   # [ST, 128, SUPER]
        flat = dots.transpose(0, 2, 1).reshape(E_PAD)  # slot order
        eid_slot = eid_slots[c]
        valid = eid_slot >= 0
        out[eid_slot[valid]] = flat[valid]
    return out, res


def kernel(h, src, dst):
    out, _ = _run(h, src, dst)
    return out
